# revision 9
# baseline (speedup 1.0000x reference)
"""Cross-channel attention kernel for Trainium2 (8 NeuronCores).

Problem (hardcoded shapes): B=2, C=64 per color -> NF=192 channels,
H=W=96 -> N=9216 spatial positions, RD=24 query/key dim.

    rgb  = concat(r,g,b)            # [B, 192, 9216]
    q    = Wq @ rgb + bq            # [B, 24, 9216]   (used as [24, j])
    k    = Wk @ rgb + bk            # [B, 24, 9216]
    v    = Wv @ rgb + bv            # [B, 192, 9216]
    attn = softmax_j(q^T k)         # [B, 9216, 9216] row-softmax over keys
    out  = rgb + v @ attn^T         # residual added on host in fp32

Sharding: data-parallel over B (2) x sequence-parallel over query rows
(4 shards of 2304) = 8 cores.  Each core gets the full rgb of its batch
(computes k and v redundantly -- they're tiny) plus its 2304 query
columns, and produces out[j, c] for its query rows.

Device-side layout trick: everything is computed "keys on partitions":
  scoresT[n, j] = sum_r k[r, n] q[r, j]        (matmul, K=r padded to 128)
  e = exp(scoresT)                             (ScalarE, PSUM -> SBUF bf16)
  acc[j, c]    += e[n, j]^T vT[n, c_aug]       (matmul, K=n chunks of 128)
where vT carries an extra all-ones column so acc[:, 192] accumulates the
softmax denominator for free; out = acc[:, :192] * (1/acc[:, 192]).
No max-subtraction is needed: logits are O(1) by construction (weights
are scaled by 0.02 in this problem), so exp() cannot overflow.

Matmul inputs are bf16 (fp32 PSUM accumulation).  The attention output
is ~0.3% of the residual magnitude, so bf16 matmul error is far below
the comparison threshold; the dominant residual term is added in fp32
exactly on the host.
"""

import numpy as np
import ml_dtypes

BF = ml_dtypes.bfloat16

# Shapes (hardcoded per problem spec)
B = 2
C = 64
HH = 96
WW = 96
N = HH * WW            # 9216 keys
NF = 3 * C             # 192 channels
RD = 24                # q/k dim
NCORES = 8
SHARDS_PER_BATCH = 4
SHARD = N // SHARDS_PER_BATCH   # 2304 query rows per core

JW = 384               # query-tile width in the attention loop
NJT = SHARD // JW      # 6 query tiles per core
PCH = 128              # key chunk (partition dim)
NCH = N // PCH         # 72 key chunks
KHI = 65               # second K-slab: channels 128..191 + ones row

_last_results = None   # BassKernelResults of the most recent run (for test.py)


def _build_program():
    import concourse.tile as tile
    from concourse import bacc, mybir

    f32 = mybir.dt.float32
    bf16 = mybir.dt.bfloat16
    Exp = mybir.ActivationFunctionType.Exp

    nc = bacc.Bacc()

    d_rgb_lo = nc.dram_tensor("rgb_lo", [128, N], bf16, kind="ExternalInput")
    d_rgb_hi = nc.dram_tensor("rgb_hi", [64, N], bf16, kind="ExternalInput")
    d_qrgb_lo = nc.dram_tensor("qrgb_lo", [128, SHARD], bf16, kind="ExternalInput")
    d_qrgb_hi = nc.dram_tensor("qrgb_hi", [64, SHARD], bf16, kind="ExternalInput")
    # weight slabs arrive pre-padded to 128 partitions (bias in row 64)
    d_wq0 = nc.dram_tensor("wq0", [128, RD], bf16, kind="ExternalInput")
    d_wq1 = nc.dram_tensor("wq1", [128, RD], bf16, kind="ExternalInput")
    d_wk0 = nc.dram_tensor("wk0", [128, RD], bf16, kind="ExternalInput")
    d_wk1 = nc.dram_tensor("wk1", [128, RD], bf16, kind="ExternalInput")
    d_wv0 = nc.dram_tensor("wv0", [128, NF + 1], bf16, kind="ExternalInput")
    d_wv1 = nc.dram_tensor("wv1", [128, NF + 1], bf16, kind="ExternalInput")
    d_out = nc.dram_tensor("out", [SHARD, NF], f32, kind="ExternalOutput")

    with tile.TileContext(nc) as tc:
        with (
            tc.tile_pool(name="const", bufs=1) as const,
            tc.tile_pool(name="work", bufs=3) as work,
            tc.tile_pool(name="pp", bufs=2, space="PSUM") as pp,
            tc.tile_pool(name="ps", bufs=2, space="PSUM") as ps,
            tc.tile_pool(name="po", bufs=4, space="PSUM") as po,
        ):
            # ---- load inputs to SBUF (pad K-slabs to 128 partitions) ----
            s_rgb_lo = const.tile([128, N], bf16)
            s_rgb_hi = const.tile([128, N], bf16)
            s_qrgb_lo = const.tile([128, SHARD], bf16)
            s_qrgb_hi = const.tile([128, SHARD], bf16)
            s_wq0 = const.tile([128, RD], bf16)
            s_wq1 = const.tile([128, RD], bf16)
            s_wk0 = const.tile([128, RD], bf16)
            s_wk1 = const.tile([128, RD], bf16)
            s_wv0 = const.tile([128, NF + 1], bf16)
            s_wv1 = const.tile([128, NF + 1], bf16)

            # rows 64:128 of the hi slabs: zeros with an all-ones row 64
            # (folds the biases in via the weight slabs' row 64).  Engine ops
            # need partition base % 32 == 0, hence [64:128] then [64:65];
            # DMAs below only touch rows 0:64 so no DMA-vs-memset deps.
            for t in (s_rgb_hi, s_qrgb_hi):
                nc.gpsimd.memset(t[64:128, :], 0.0)
                nc.gpsimd.memset(t[64:65, :], 1.0)

            # split the big DMAs column-wise to spread over queues
            nsplit = 4
            for i in range(nsplit):
                sl = slice(i * (N // nsplit), (i + 1) * (N // nsplit))
                nc.sync.dma_start(out=s_rgb_lo[:, sl], in_=d_rgb_lo[:, sl])
                nc.sync.dma_start(out=s_rgb_hi[:64, sl], in_=d_rgb_hi[:, sl])
            for i in range(2):
                sl = slice(i * (SHARD // 2), (i + 1) * (SHARD // 2))
                nc.sync.dma_start(out=s_qrgb_lo[:, sl], in_=d_qrgb_lo[:, sl])
                nc.sync.dma_start(out=s_qrgb_hi[:64, sl], in_=d_qrgb_hi[:, sl])
            nc.sync.dma_start(out=s_wq0[:], in_=d_wq0[:])
            nc.sync.dma_start(out=s_wq1[:], in_=d_wq1[:])
            nc.sync.dma_start(out=s_wk0[:], in_=d_wk0[:])
            nc.sync.dma_start(out=s_wk1[:], in_=d_wk1[:])
            nc.sync.dma_start(out=s_wv0[:], in_=d_wv0[:])
            nc.sync.dma_start(out=s_wv1[:], in_=d_wv1[:])

            # ---- projections ----
            # k[r, n] for all keys; q[r, j] for this shard (K-contraction over
            # channels, split 128 + 65-padded-to-128).  Rows RD..128 of s_k /
            # s_q are zero so the attention matmul can contract over K=128.
            s_k = const.tile([128, N], bf16)
            s_q = const.tile([128, SHARD], bf16)
            # zero everything (partition base of a memset must be 32-aligned,
            # so rows 24:128 alone can't be targeted); projection copies then
            # overwrite rows 0:24
            nc.gpsimd.memset(s_k[:, :], 0.0)
            nc.gpsimd.memset(s_q[:, :], 0.0)

            KT = 512
            for t in range(N // KT):
                sl = slice(t * KT, (t + 1) * KT)
                pk = pp.tile([128, KT], f32, tag="pp", name=f"pk_{t}")
                nc.tensor.matmul(pk[:RD, :], lhsT=s_wk0, rhs=s_rgb_lo[:, sl],
                                 start=True, stop=False)
                nc.tensor.matmul(pk[:RD, :], lhsT=s_wk1, rhs=s_rgb_hi[:, sl],
                                 start=False, stop=True)
                nc.vector.tensor_copy(out=s_k[:RD, sl], in_=pk[:RD, :])
            QT = 384
            for t in range(SHARD // QT):
                sl = slice(t * QT, (t + 1) * QT)
                pq = pp.tile([128, QT], f32, tag="pp", name=f"pq_{t}")
                nc.tensor.matmul(pq[:RD, :], lhsT=s_wq0, rhs=s_qrgb_lo[:, sl],
                                 start=True, stop=False)
                nc.tensor.matmul(pq[:RD, :], lhsT=s_wq1, rhs=s_qrgb_hi[:, sl],
                                 start=False, stop=True)
                nc.vector.tensor_copy(out=s_q[:RD, sl], in_=pq[:RD, :])

            # vT[n, c] (+ ones column) for all keys, chunked by 128 keys
            s_vT = const.tile([128, NCH, NF + 1], bf16)
            for t in range(NCH):
                sl = slice(t * PCH, (t + 1) * PCH)
                pv = pp.tile([128, NF + 1], f32, tag="pp", name=f"pv_{t}")
                nc.tensor.matmul(pv, lhsT=s_rgb_lo[:, sl], rhs=s_wv0,
                                 start=True, stop=False)
                nc.tensor.matmul(pv, lhsT=s_rgb_hi[:, sl], rhs=s_wv1,
                                 start=False, stop=True)
                nc.vector.tensor_copy(out=s_vT[:, t, :], in_=pv)

            # ---- attention ----
            for jt in range(NJT):
                q_sl = s_q[:, jt * JW:(jt + 1) * JW]
                acc = [po.tile([128, NF + 1], f32, tag="po", name=f"acc_{jt}_{s}")
                       for s in range(JW // 128)]

                # software-pipelined: scores(nck) runs on PE while exp(nck-1)
                # finishes on ScalarE, then the nck-1 accumulation matmuls.
                e_prev = None

                def accum(e_tile, nck):
                    for s in range(JW // 128):
                        nc.tensor.matmul(
                            acc[s],
                            lhsT=e_tile[:, s * 128:(s + 1) * 128],
                            rhs=s_vT[:, nck, :],
                            start=(nck == 0), stop=(nck == NCH - 1),
                        )

                for nck in range(NCH):
                    pst = ps.tile([128, JW], f32, tag="ps", name=f"ps_{jt}_{nck}")
                    nc.tensor.matmul(pst,
                                     lhsT=s_k[:, nck * PCH:(nck + 1) * PCH],
                                     rhs=q_sl, start=True, stop=True)
                    e_t = work.tile([128, JW], bf16, tag="e", name=f"e_{jt}_{nck}")
                    nc.scalar.activation(out=e_t, in_=pst, func=Exp)
                    if e_prev is not None:
                        accum(e_prev, nck - 1)
                    e_prev = e_t
                accum(e_prev, NCH - 1)

                for s in range(JW // 128):
                    rec = work.tile([128, 1], f32, tag="rec", name=f"rec_{jt}_{s}")
                    nc.vector.reciprocal(rec, acc[s][:, NF:NF + 1])
                    o_sb = work.tile([128, NF], f32, tag="osb", name=f"o_{jt}_{s}")
                    nc.vector.tensor_scalar_mul(o_sb, acc[s][:, 0:NF], rec)
                    r0 = jt * JW + s * 128
                    nc.sync.dma_start(out=d_out[r0:r0 + 128, :], in_=o_sb)

    nc.compile()
    return nc


def kernel(r, g, b, Wq, bq, Wk, bk, Wv, bv):
    global _last_results
    from concourse.bass_utils import run_bass_kernel_spmd

    r = np.asarray(r, np.float32)
    g = np.asarray(g, np.float32)
    b = np.asarray(b, np.float32)
    Wq = np.asarray(Wq, np.float32)
    bq = np.asarray(bq, np.float32)
    Wk = np.asarray(Wk, np.float32)
    bk = np.asarray(bk, np.float32)
    Wv = np.asarray(Wv, np.float32)
    bv = np.asarray(bv, np.float32)

    rgb = np.concatenate([r, g, b], axis=1).reshape(B, NF, N)  # fp32

    def bf(a):
        return np.ascontiguousarray(a).astype(BF)

    WqT = Wq.T  # [192, 24]
    WkT = Wk.T
    WvT = Wv.T  # [192, 192]

    def pad_hi(w_hi, bias_row):
        # [64 rows of W.T | bias row | zeros] -> [128, cols]
        out = np.zeros((128, w_hi.shape[1]), np.float32)
        out[:64] = w_hi
        out[64] = bias_row
        return bf(out)

    wq0 = bf(WqT[:128])
    wq1 = pad_hi(WqT[128:], bq)
    wk0 = bf(WkT[:128])
    wk1 = pad_hi(WkT[128:], bk)
    wv0 = bf(np.concatenate([WvT[:128], np.zeros((128, 1), np.float32)], axis=1))
    wv1 = pad_hi(np.concatenate([WvT[128:], np.zeros((64, 1), np.float32)], axis=1),
                 np.concatenate([bv, np.ones(1, np.float32)]))

    in_maps = []
    for core in range(NCORES):
        bi = core // SHARDS_PER_BATCH
        j0 = (core % SHARDS_PER_BATCH) * SHARD
        rgb_b = rgb[bi]
        in_maps.append({
            "rgb_lo": bf(rgb_b[:128]),
            "rgb_hi": bf(rgb_b[128:]),
            "qrgb_lo": bf(rgb_b[:128, j0:j0 + SHARD]),
            "qrgb_hi": bf(rgb_b[128:, j0:j0 + SHARD]),
            "wq0": wq0, "wq1": wq1,
            "wk0": wk0, "wk1": wk1,
            "wv0": wv0, "wv1": wv1,
        })

    nc = _build_program()
    res = run_bass_kernel_spmd(nc, in_maps, list(range(NCORES)))
    _last_results = res

    att = np.empty((B, N, NF), np.float32)
    for core in range(NCORES):
        bi = core // SHARDS_PER_BATCH
        j0 = (core % SHARDS_PER_BATCH) * SHARD
        att[bi, j0:j0 + SHARD, :] = res.results[core]["out"]

    out = rgb + att.transpose(0, 2, 1)          # fp32 residual, exact
    out = out.reshape(B, NF, HH, WW)
    return (out[:, :C], out[:, C:2 * C], out[:, 2 * C:])
